# revision 1
# baseline (speedup 1.0000x reference)
"""Trainium2 Bass kernel for nn_BaseLinearSSM.

y[b,t] = Re(C @ x_{t+1}) + D @ u[b,t] + bias,  x_{t+1} = A x_t + B u_t  (complex A,B,C)

Strategy:
  Host (fp64): eigendecompose A = V diag(w) V^-1  (cond(V) ~ 370 for this
  problem class), fold V into B/C:  Bt = V^-1 B, Ct = C V.  The recurrence
  becomes diagonal:  xt_{t+1} = w * xt_t + Bt u_t.  Writing w = rho*e^{i th},
  z_t = e^{-i th t} xt_t obeys  z_t = rho * z_{t-1} + e^{-i th t} (Bt u)_t --
  two *real* first-order scans per mode, which map 1:1 onto the DVE's native
  tensor_tensor_scan (state = data0*state + data1).

  Device (per core, batch-sharded 2 of 16):
    f = Bt^T-matmuls of u  ->  modulate by cos/sin(th*t) tables (host fp64)
    -> tensor_tensor_scan along t  ->  demodulate  ->  y = CtRe.x_r - CtIm.x_i
    + D u accumulated in one PSUM group.

  Cores are fully independent (A/B/C/D replicated); host shards u and
  gathers y.
"""

import sys

import numpy as np

if "/opt/trn_rl_repo" not in sys.path:
    sys.path.insert(0, "/opt/trn_rl_repo")

BATCH, T, IN, OUT, N = 16, 2048, 128, 128, 512
NCORES = 8
BLOCAL = BATCH // NCORES  # 2
COLS = BLOCAL * T         # 4096 columns per core, col = b*T + t
NT = N // 128             # 4 partition tiles over the state dim
BLK = 512                 # columns per pipeline block
NBLK = COLS // BLK        # 8 blocks, (b, tb) with tb in 0..3
TBLK = T // BLK           # 4 t-blocks per batch element
# blob pieces (also DMA issue order):
#   p0: ut | btr | bti          (gates the f-matmuls)
#   p1: tb0 tables              (gates the first modulate)
#   p2: dwt | ctr | cti | rho   (gates y-projection / scans)
#   p3..p5: tb1..tb3 tables
P0W = COLS + N + N
TBW = 2 * NT * BLK  # one tb's cos+sin tables
P2W = OUT + NT * OUT + NT * OUT + NT * BLK
BLOBW = P0W + P2W + TBLK * TBW

LAST_RESULT = None  # BassKernelResults of the most recent run (for profiling)

_NC_CACHE = None


def _build_nc():
    """Build the SPMD Bass program (identical on all 8 cores)."""
    from concourse import bass, mybir
    from concourse import tile

    dt = mybir.dt.float32
    op = mybir.AluOpType

    nc = bass.Bass("TRN2", target_bir_lowering=False, debug=False)

    # All inputs packed in ONE [128, W] blob -> one DMA -> one HW queue ->
    # at most one DMA sync wait on any consumer (fused fp32 LDW+MATMUL
    # supports a single sync wait).
    blob = nc.dram_tensor("blob", [128, BLOBW], dt, kind="ExternalInput")
    yout = nc.dram_tensor("y", [OUT, COLS], dt, kind="ExternalOutput")  # [o, b*T+t]

    with tile.TileContext(nc) as tc:
        with (
            tc.tile_pool(name="const", bufs=1) as cpool,
            tc.tile_pool(name="tmp", bufs=2) as tpool,
            tc.tile_pool(name="gp", bufs=1) as gpool,
            tc.tile_pool(name="zp", bufs=2) as zpool,
            tc.tile_pool(name="xr", bufs=1) as xrpool,
            tc.tile_pool(name="xi", bufs=2) as xipool,
            tc.tile_pool(name="ysb", bufs=2) as spool,
            tc.tile_pool(name="fps", bufs=6, space="PSUM") as fpool,
            tc.tile_pool(name="yps", bufs=2, space="PSUM") as ypool,
        ):
            blob_sb = cpool.tile([128, BLOBW], dt)
            # Issue order = dependency order: f-matmul inputs, first tables,
            # projection weights, remaining tables.  _legalize_multi_waits
            # keeps any resulting wait pairing legal for walrus.
            bounds = [0, P0W, P0W + TBW, P0W + TBW + P2W]
            for k in range(2, TBLK + 1):
                bounds.append(bounds[-1] + TBW)
            for a, bnd in zip(bounds[:-1], bounds[1:]):
                nc.sync.dma_start(blob_sb[:, a:bnd], blob[:, a:bnd])
            o = [0]
            def take(w):
                s = blob_sb[:, o[0]:o[0] + w]
                o[0] += w
                return s
            ut_sb = take(COLS)
            btr_sb = take(N)
            bti_sb = take(N)
            ct_tb = [[None] * NT for _ in range(TBLK)]
            st_tb = [[None] * NT for _ in range(TBLK)]
            for m in range(NT):
                ct_tb[0][m] = take(BLK)
            for m in range(NT):
                st_tb[0][m] = take(BLK)
            dwt_sb = take(OUT)
            ctr_sb = take(NT * OUT)
            cti_sb = take(NT * OUT)
            rho_sb = [take(BLK) for _ in range(NT)]
            for k in range(1, TBLK):
                for m in range(NT):
                    ct_tb[k][m] = take(BLK)
                for m in range(NT):
                    st_tb[k][m] = take(BLK)
            assert o[0] == BLOBW

            zr_prev = [None] * NT
            zi_prev = [None] * NT
            for b in range(BLOCAL):
                for tb in range(TBLK):
                    col0 = b * T + tb * BLK
                    ucols = ut_sb[:, col0:col0 + BLK]
                    xr_blk = [None] * NT
                    xi_blk = [None] * NT
                    for m in range(NT):
                        ctt = ct_tb[tb][m][:]
                        stt = st_tb[tb][m][:]
                        # f = Bt u  (complex), PSUM
                        fre = fpool.tile([128, BLK], dt, tag="f")
                        fim = fpool.tile([128, BLK], dt, tag="f")
                        nc.tensor.matmul(
                            fre[:], btr_sb[:, m * 128:(m + 1) * 128], ucols
                        )
                        nc.tensor.matmul(
                            fim[:], bti_sb[:, m * 128:(m + 1) * 128], ucols
                        )
                        # modulate: g = e^{-i th t} f
                        t1 = tpool.tile([128, BLK], dt, tag="t1")
                        t2 = tpool.tile([128, BLK], dt, tag="t2")
                        nc.vector.tensor_tensor(t1[:], ctt, fre[:], op=op.mult)
                        nc.vector.tensor_tensor(t2[:], stt, fim[:], op=op.mult)
                        gr = gpool.tile([128, BLK], dt, tag=f"gr{m}")
                        nc.vector.tensor_tensor(gr[:], t1[:], t2[:], op=op.add)
                        t3 = tpool.tile([128, BLK], dt, tag="t1")
                        t4 = tpool.tile([128, BLK], dt, tag="t2")
                        nc.vector.tensor_tensor(t3[:], ctt, fim[:], op=op.mult)
                        nc.vector.tensor_tensor(t4[:], stt, fre[:], op=op.mult)
                        gi = gpool.tile([128, BLK], dt, tag=f"gi{m}")
                        nc.vector.tensor_tensor(gi[:], t3[:], t4[:], op=op.subtract)
                        # scan: z = rho*z_prev + g along t (chained across tb)
                        zr = zpool.tile([128, BLK], dt, tag=f"zr{m}")
                        zi = zpool.tile([128, BLK], dt, tag=f"zi{m}")
                        init_r = 0.0 if tb == 0 else zr_prev[m][:, BLK - 1:BLK]
                        init_i = 0.0 if tb == 0 else zi_prev[m][:, BLK - 1:BLK]
                        nc.vector.tensor_tensor_scan(
                            zr[:], rho_sb[m][:], gr[:], init_r, op0=op.mult, op1=op.add
                        )
                        nc.vector.tensor_tensor_scan(
                            zi[:], rho_sb[m][:], gi[:], init_i, op0=op.mult, op1=op.add
                        )
                        zr_prev[m], zi_prev[m] = zr, zi
                        # demodulate: x = e^{i th t} z
                        t5 = tpool.tile([128, BLK], dt, tag="t1")
                        t6 = tpool.tile([128, BLK], dt, tag="t2")
                        nc.vector.tensor_tensor(t5[:], ctt, zr[:], op=op.mult)
                        nc.vector.tensor_tensor(t6[:], stt, zi[:], op=op.mult)
                        xr = xrpool.tile([128, BLK], dt, tag=f"xr{m}")
                        nc.vector.tensor_tensor(xr[:], t5[:], t6[:], op=op.subtract)
                        t7 = tpool.tile([128, BLK], dt, tag="t7")
                        t8 = tpool.tile([128, BLK], dt, tag="t8")
                        nc.gpsimd.tensor_tensor(t7[:], stt, zr[:], op=op.mult)
                        nc.gpsimd.tensor_tensor(t8[:], ctt, zi[:], op=op.mult)
                        xi = xipool.tile([128, BLK], dt, tag=f"xi{m}")
                        nc.gpsimd.tensor_tensor(xi[:], t7[:], t8[:], op=op.add)
                        xr_blk[m], xi_blk[m] = xr, xi
                    # y = sum_m CtRe_m^T x_r[m] + (-CtIm_m)^T x_i[m] + D^T u
                    yps = ypool.tile([128, BLK], dt, tag="y")
                    for m in range(NT):
                        nc.tensor.matmul(
                            yps[:], ctr_sb[:, m * OUT:(m + 1) * OUT], xr_blk[m][:],
                            start=(m == 0), stop=False,
                        )
                        nc.tensor.matmul(
                            yps[:], cti_sb[:, m * OUT:(m + 1) * OUT], xi_blk[m][:],
                            start=False, stop=False,
                        )
                    nc.tensor.matmul(
                        yps[:], dwt_sb[:], ucols, start=False, stop=True
                    )
                    ysb = spool.tile([128, BLK], dt, tag="ysb")
                    nc.scalar.copy(ysb[:], yps[:])
                    nc.gpsimd.dma_start(yout[:, col0:col0 + BLK], ysb[:])

    _legalize_multi_waits(nc)
    return nc


def _legalize_multi_waits(nc):
    """This walrus build accepts a single sync wait per instruction; split
    any multi-wait instruction into same-engine single-wait NoOps + the
    original carrying the last wait (program order chains them)."""
    import bass_rust
    from concourse import mybir

    uid = [0]
    for fn in nc.m.functions:
        for bb in fn.blocks:
            insts = bb.instructions
            new = []
            changed = False
            for inst in insts:
                si = inst.sync_info
                if si is not None and len(si.on_wait) > 1:
                    waits = list(si.on_wait)
                    for w in waits[:-1]:
                        uid[0] += 1
                        new.append(mybir.InstNoOp(
                            name=f"mwsplit-{uid[0]}",
                            engine=inst.engine,
                            ins=[], outs=[],
                            sync_info=bass_rust.SyncInfo(on_wait=[w], on_update=[]),
                        ))
                    inst.sync_info = bass_rust.SyncInfo(
                        on_wait=[waits[-1]], on_update=list(si.on_update)
                    )
                    changed = True
                new.append(inst)
            if changed:
                bb.instructions = new


def _host_prep(A_re, A_im, B_re, B_im, C_re, C_im, D_w):
    """fp64 eigendecomposition + transposed/modulation-table layouts."""
    A = A_re.astype(np.float64) + 1j * A_im.astype(np.float64)
    w, V = np.linalg.eig(A)
    Vinv = np.linalg.inv(V)
    Bt = Vinv @ (B_re.astype(np.float64) + 1j * B_im.astype(np.float64))  # [N, IN]
    Ct = (C_re.astype(np.float64) + 1j * C_im.astype(np.float64)) @ V     # [OUT, N]

    rho = np.abs(w)
    theta = np.angle(w)
    tg = np.arange(1, T + 1, dtype=np.float64)
    ang = np.outer(theta, tg)  # [N, T]
    cost = np.cos(ang).astype(np.float32).reshape(NT, 128, T)
    sint = np.sin(ang).astype(np.float32).reshape(NT, 128, T)
    rho_b = np.broadcast_to(
        rho.astype(np.float32).reshape(NT, 128, 1), (NT, 128, BLK)
    ).copy()

    ctrT = np.ascontiguousarray(Ct.real.T, dtype=np.float32)   # [N, OUT]
    ctiT = np.ascontiguousarray(-Ct.imag.T, dtype=np.float32)  # [N, OUT]
    # shared blob columns (everything except the leading per-core ut block),
    # all [128, w]:
    def tbpiece(k):
        cs = cost[:, :, k * BLK:(k + 1) * BLK]  # [NT, 128, BLK]
        ss = sint[:, :, k * BLK:(k + 1) * BLK]
        return [np.ascontiguousarray(cs.transpose(1, 0, 2).reshape(128, NT * BLK)),
                np.ascontiguousarray(ss.transpose(1, 0, 2).reshape(128, NT * BLK))]
    parts = [
        np.ascontiguousarray(Bt.real.T, dtype=np.float32),  # [128(i), N]
        np.ascontiguousarray(Bt.imag.T, dtype=np.float32),
    ]
    parts += tbpiece(0)
    parts += [np.ascontiguousarray(D_w.T, dtype=np.float32)]
    parts += [np.ascontiguousarray(ctrT.reshape(NT, 128, OUT).transpose(1, 0, 2)
                                   .reshape(128, NT * OUT))]
    parts += [np.ascontiguousarray(ctiT.reshape(NT, 128, OUT).transpose(1, 0, 2)
                                   .reshape(128, NT * OUT))]
    parts += [np.ascontiguousarray(rho_b.transpose(1, 0, 2).reshape(128, NT * BLK))]
    for k in range(1, TBLK):
        parts += tbpiece(k)
    return np.concatenate(parts, axis=1)  # [128, BLOBW - COLS]


def _ensure_axon_hooks():
    """Provide antenv.axon_hooks if the image lacks it (needed only for
    trace=True NTFF profiling; run path works without)."""
    import types
    try:
        from antenv import axon_hooks  # noqa: F401
        return
    except ImportError:
        pass
    try:
        import antenv
        mod = types.ModuleType("antenv.axon_hooks")
        _hook = [None]
        mod.set_axon_ntff_profile_hook = lambda h: _hook.__setitem__(0, h)
        mod.get_axon_ntff_profile_hook = lambda: _hook[0]
        sys.modules["antenv.axon_hooks"] = mod
        antenv.axon_hooks = mod
        if "/root/.axon_site" not in sys.path:
            sys.path.insert(0, "/root/.axon_site")
        from trn_agent_boot.trn_boot import _ntff_profile_via_ctypes
        h = _ntff_profile_via_ctypes("/opt/axon/libaxon_pjrt.so")
        if h is not None:
            mod.set_axon_ntff_profile_hook(h)
    except Exception:
        pass


def kernel(u, A_re, A_im, B_re, B_im, C_re, C_im, D_w, output_bias):
    global LAST_RESULT, _NC_CACHE
    from concourse import bass_utils

    _ensure_axon_hooks()

    u = np.asarray(u, dtype=np.float32)
    shared = _host_prep(
        np.asarray(A_re), np.asarray(A_im), np.asarray(B_re), np.asarray(B_im),
        np.asarray(C_re), np.asarray(C_im), np.asarray(D_w)
    )

    if _NC_CACHE is None:
        _NC_CACHE = _build_nc()
    nc = _NC_CACHE

    in_maps = []
    for k in range(NCORES):
        u_pair = u[BLOCAL * k:BLOCAL * (k + 1)]  # [2, T, IN]
        ut = np.ascontiguousarray(
            u_pair.transpose(2, 0, 1).reshape(128, COLS), dtype=np.float32
        )
        in_maps.append({"blob": np.concatenate([ut, shared], axis=1)})

    res = bass_utils.run_bass_kernel_spmd(nc, in_maps, core_ids=list(range(NCORES)))
    LAST_RESULT = res

    y = np.empty((BATCH, T, OUT), dtype=np.float32)
    for k in range(NCORES):
        yd = res.results[k]["y"]  # [OUT, COLS]
        y[BLOCAL * k:BLOCAL * (k + 1)] = (
            yd.reshape(OUT, BLOCAL, T).transpose(1, 2, 0)
        )
    y += np.asarray(output_bias, dtype=np.float32)
    return y



# revision 2
# speedup vs baseline: 2.4837x; 2.4837x over previous
"""Trainium2 Bass kernel for nn_BaseLinearSSM.

y[b,t] = Re(C @ x_{t+1}) + D @ u[b,t] + bias,  x_{t+1} = A x_t + B u_t  (complex A,B,C)

Strategy (v2, hybrid fp16):
  Host (fp64): eigendecompose A = V diag(w) V^-1, fold V into B/C:
  Bt = V^-1 B, Ct = C V.  Sort modes by |w| descending and split:

  * LONG modes (top NL=256, two 128-partition tiles, per-mode balanced so
    ||bt_n|| = 1): modulated real scans exactly as v1 --
    f = Bt^T u -> g = e^{-i th t} f -> tensor_tensor_scan(rho) -> demod
    products -> y matmuls.  All elementwise tensors are fp16 so DVE runs
    in 2x mode; scans keep an fp32 rho and internal fp32 state.

  * SHORT modes (|w| < ~0.64): their memory is ~16 steps, so their whole
    contribution (plus the D term at lag 0) is a 16-lag convolution
    y += sum_k Re(Ct w^k Bt) u_{t-k}, computed entirely on the tensor
    engine with fp16 [128,128] kernels accumulating into the y PSUM bank.

  Numerics validated host-side: fp16 hybrid rel err ~3e-3 (gate 2e-2).

  Cores are fully independent (weights replicated); host shards u (2 of
  16 batch per core) and gathers y (fp16 out, converted + bias on host).
"""

import sys

import numpy as np

if "/opt/trn_rl_repo" not in sys.path:
    sys.path.insert(0, "/opt/trn_rl_repo")

BATCH, T, IN, OUT, N = 16, 2048, 128, 128, 512
NCORES = 8
BLOCAL = BATCH // NCORES  # 2
COLS = BLOCAL * T         # 4096 columns per core, col = b*T + t
BLK = 512                 # columns per pipeline block
NBLK = COLS // BLK        # 8 blocks
TBLK = T // BLK           # 4 t-blocks per batch element
NL = 256                  # long modes (scan path)
LT = NL // 128            # 2 long-mode partition tiles
KCONV = 16                # conv lags for short modes (incl. D at lag 0)

# fp16 blob layout (widths in fp16 elements), issue order = first-use order:
#   ut | convW | btr | bti | tb0(c|s) | ctrT | ctiT | tb1 | tb2 | tb3
W_UT = COLS
W_CONV = KCONV * OUT
W_B = LT * 128
TBW = 2 * LT * BLK        # one tb's cos+sin tables
W_CT = LT * OUT
BLOBW = W_UT + W_CONV + 2 * W_B + 2 * W_CT + TBLK * TBW
RHOW = LT * BLK           # fp32 rho table width

# engine assignment for elementwise sites (tune): 'v' = vector, 'g' = gpsimd
MOD_ENGINES = ["v", "v", "v", "v", "v", "v"]   # t1,t2,gr,t3,t4,gi
SCAN_ENGINES = ["v", "v", "v", "v"]            # zr0, zi0, zr1, zi1
DEM_ENGINES = ["v", "v", "v", "v"]             # p,q,r,w2
CMB_ENGINES = ["g", "g"]                       # pr = p-r, qw = q+w2

LAST_RESULT = None
_NC_CACHE = None


def _build_nc():
    from concourse import bass, mybir
    from concourse import tile

    f16 = mybir.dt.float16
    f32 = mybir.dt.float32
    op = mybir.AluOpType

    nc = bass.Bass("TRN2", target_bir_lowering=False, debug=False)

    blob = nc.dram_tensor("blob", [128, BLOBW], f16, kind="ExternalInput")
    rhod = nc.dram_tensor("rho", [128, RHOW], f32, kind="ExternalInput")
    yout = nc.dram_tensor("y", [OUT, COLS], f16, kind="ExternalOutput")

    with tile.TileContext(nc) as tc:
        with (
            tc.tile_pool(name="const", bufs=1) as cpool,
            tc.tile_pool(name="rho", bufs=1) as rpool,
            tc.tile_pool(name="f16", bufs=2) as fspool,
            tc.tile_pool(name="tmp", bufs=2) as tpool,
            tc.tile_pool(name="gp", bufs=2) as gpool,
            tc.tile_pool(name="zp", bufs=2) as zpool,
            tc.tile_pool(name="dm", bufs=2) as dpool,
            tc.tile_pool(name="cmb", bufs=2) as mpool,
            tc.tile_pool(name="ysb", bufs=2) as spool,
            tc.tile_pool(name="fps", bufs=1, space="PSUM") as fpool,
            tc.tile_pool(name="yps", bufs=2, space="PSUM") as ypool,
        ):
            blob_sb = cpool.tile([128, BLOBW], f16)
            rho_sb = rpool.tile([128, RHOW], f32)
            # chunked DMA in first-use order
            bounds = [0, W_UT + W_CONV, W_UT + W_CONV + 2 * W_B]
            bounds.append(bounds[-1] + TBW)
            bounds.append(bounds[-1] + 2 * W_CT)
            for _ in range(1, TBLK):
                bounds.append(bounds[-1] + TBW)
            for a, bnd in zip(bounds[:-1], bounds[1:]):
                nc.sync.dma_start(blob_sb[:, a:bnd], blob[:, a:bnd])
            nc.sync.dma_start(rho_sb[:], rhod[:])

            o = [0]
            def take(w):
                s = blob_sb[:, o[0]:o[0] + w]
                o[0] += w
                return s
            ut_sb = take(W_UT)
            convw = take(W_CONV)
            btr_sb = take(W_B)
            bti_sb = take(W_B)
            ct_tb = [None] * TBLK
            st_tb = [None] * TBLK
            ct_tb[0] = take(LT * BLK)
            st_tb[0] = take(LT * BLK)
            ctr_sb = take(W_CT)
            cti_sb = take(W_CT)
            for k in range(1, TBLK):
                ct_tb[k] = take(LT * BLK)
                st_tb[k] = take(LT * BLK)
            assert o[0] == BLOBW

            eng = {"v": nc.vector, "g": nc.gpsimd}

            def emit_conv_f(j):
                """PE: conv matmuls + f matmuls for block j; returns psum tiles."""
                b, tb = divmod(j, TBLK)
                col0 = b * T + tb * BLK
                yps = ypool.tile([128, BLK], f32, tag="y")
                for k in range(KCONV):
                    wk = convw[:, k * OUT:(k + 1) * OUT]
                    if tb == 0 and k > 0:
                        nc.tensor.matmul(
                            yps[:, k:BLK], wk, ut_sb[:, col0:col0 + BLK - k],
                            start=False, stop=False,
                        )
                    else:
                        nc.tensor.matmul(
                            yps[:], wk, ut_sb[:, col0 - k:col0 + BLK - k],
                            start=(k == 0), stop=False,
                        )
                ucols = ut_sb[:, col0:col0 + BLK]
                fps = []
                for m in range(LT):
                    fre = fpool.tile([128, BLK], f32, tag=f"fr{m}")
                    fim = fpool.tile([128, BLK], f32, tag=f"fi{m}")
                    nc.tensor.matmul(fre[:], btr_sb[:, m * 128:(m + 1) * 128], ucols)
                    nc.tensor.matmul(fim[:], bti_sb[:, m * 128:(m + 1) * 128], ucols)
                    fps.append((fre, fim))
                return yps, ucols, fps

            def emit_evict_f(fps):
                """ACT: evict f PSUM -> one wide fp16 SBUF pair."""
                fr16 = fspool.tile([128, LT * BLK], f16, tag="fr16")
                fi16 = fspool.tile([128, LT * BLK], f16, tag="fi16")
                for m, (fre, fim) in enumerate(fps):
                    nc.scalar.copy(fr16[:, m * BLK:(m + 1) * BLK], fre[:])
                    nc.scalar.copy(fi16[:, m * BLK:(m + 1) * BLK], fim[:])
                return fr16, fi16

            zprev = [None, None]  # (zr, zi) wide tiles of previous block

            def emit_dve(j, fr16, fi16):
                b, tb = divmod(j, TBLK)
                ctt, stt = ct_tb[tb][:], st_tb[tb][:]
                W = LT * BLK
                # modulate (wide over both tiles)
                t1 = tpool.tile([128, W], f16, tag="t1")
                t2 = tpool.tile([128, W], f16, tag="t2")
                gr = gpool.tile([128, W], f16, tag="gr")
                eng[MOD_ENGINES[0]].tensor_tensor(t1[:], ctt, fr16[:], op=op.mult)
                eng[MOD_ENGINES[1]].tensor_tensor(t2[:], stt, fi16[:], op=op.mult)
                eng[MOD_ENGINES[2]].tensor_tensor(gr[:], t1[:], t2[:], op=op.add)
                t3 = tpool.tile([128, W], f16, tag="t3")
                t4 = tpool.tile([128, W], f16, tag="t4")
                gi = gpool.tile([128, W], f16, tag="gi")
                eng[MOD_ENGINES[3]].tensor_tensor(t3[:], ctt, fi16[:], op=op.mult)
                eng[MOD_ENGINES[4]].tensor_tensor(t4[:], stt, fr16[:], op=op.mult)
                eng[MOD_ENGINES[5]].tensor_tensor(gi[:], t3[:], t4[:], op=op.subtract)
                # scans (per tile, chained across tb within each batch elem)
                zr = zpool.tile([128, W], f16, tag="zr")
                zi = zpool.tile([128, W], f16, tag="zi")
                for m in range(LT):
                    sl = slice(m * BLK, (m + 1) * BLK)
                    init_r = 0.0 if tb == 0 else zprev[0][:, m * BLK + BLK - 1:m * BLK + BLK]
                    init_i = 0.0 if tb == 0 else zprev[1][:, m * BLK + BLK - 1:m * BLK + BLK]
                    eng[SCAN_ENGINES[2 * m]].tensor_tensor_scan(
                        zr[:, sl], rho_sb[:, sl], gr[:, sl], init_r,
                        op0=op.mult, op1=op.add,
                    )
                    eng[SCAN_ENGINES[2 * m + 1]].tensor_tensor_scan(
                        zi[:, sl], rho_sb[:, sl], gi[:, sl], init_i,
                        op0=op.mult, op1=op.add,
                    )
                zprev[0], zprev[1] = zr, zi
                # demod products
                p = dpool.tile([128, W], f16, tag="p")
                q = dpool.tile([128, W], f16, tag="q")
                r = dpool.tile([128, W], f16, tag="r")
                w2 = dpool.tile([128, W], f16, tag="w2")
                eng[DEM_ENGINES[0]].tensor_tensor(p[:], ctt, zr[:], op=op.mult)
                eng[DEM_ENGINES[1]].tensor_tensor(q[:], stt, zr[:], op=op.mult)
                eng[DEM_ENGINES[2]].tensor_tensor(r[:], stt, zi[:], op=op.mult)
                eng[DEM_ENGINES[3]].tensor_tensor(w2[:], ctt, zi[:], op=op.mult)
                # combine: pr = p - r, qw = q + w2
                pr = mpool.tile([128, W], f16, tag="pr")
                qw = mpool.tile([128, W], f16, tag="qw")
                eng[CMB_ENGINES[0]].tensor_tensor(pr[:], p[:], r[:], op=op.subtract)
                eng[CMB_ENGINES[1]].tensor_tensor(qw[:], q[:], w2[:], op=op.add)
                return pr, qw

            def emit_y(j, yps, pr, qw):
                b, tb = divmod(j, TBLK)
                col0 = b * T + tb * BLK
                for m in range(LT):
                    sl = slice(m * BLK, (m + 1) * BLK)
                    nc.tensor.matmul(
                        yps[:], ctr_sb[:, m * OUT:(m + 1) * OUT], pr[:, sl],
                        start=False, stop=False,
                    )
                    nc.tensor.matmul(
                        yps[:], cti_sb[:, m * OUT:(m + 1) * OUT], qw[:, sl],
                        start=False, stop=(m == LT - 1),
                    )
                ysb = spool.tile([128, BLK], f16, tag="ysb")
                nc.scalar.copy(ysb[:], yps[:])
                nc.gpsimd.dma_start(yout[:, col0:col0 + BLK], ysb[:])

            # software-pipelined emission
            yps_c, uc_c, fps_c = emit_conv_f(0)
            ev_c = emit_evict_f(fps_c)
            for j in range(NBLK):
                if j + 1 < NBLK:
                    yps_n, uc_n, fps_n = emit_conv_f(j + 1)
                    ev_n = emit_evict_f(fps_n)
                pr, qw = emit_dve(j, *ev_c)
                emit_y(j, yps_c, pr, qw)
                if j + 1 < NBLK:
                    yps_c, uc_c, fps_c = yps_n, uc_n, fps_n
                    ev_c = ev_n

    _legalize_multi_waits(nc)
    return nc


def _legalize_multi_waits(nc):
    """This walrus build accepts a single sync wait per instruction; split
    any multi-wait instruction into same-engine single-wait NoOps + the
    original carrying the last wait (program order chains them)."""
    import bass_rust
    from concourse import mybir

    uid = [0]
    for fn in nc.m.functions:
        for bb in fn.blocks:
            insts = bb.instructions
            new = []
            changed = False
            for inst in insts:
                si = inst.sync_info
                if si is not None and len(si.on_wait) > 1:
                    waits = list(si.on_wait)
                    for w in waits[:-1]:
                        uid[0] += 1
                        new.append(mybir.InstNoOp(
                            name=f"mwsplit-{uid[0]}",
                            engine=inst.engine,
                            ins=[], outs=[],
                            sync_info=bass_rust.SyncInfo(on_wait=[w], on_update=[]),
                        ))
                    inst.sync_info = bass_rust.SyncInfo(
                        on_wait=[waits[-1]], on_update=list(si.on_update)
                    )
                    changed = True
                new.append(inst)
            if changed:
                bb.instructions = new


def _host_prep(A_re, A_im, B_re, B_im, C_re, C_im, D_w):
    """fp64 eigendecomposition, mode sort/split, fp16 table/weight layouts."""
    A = A_re.astype(np.float64) + 1j * A_im.astype(np.float64)
    w, V = np.linalg.eig(A)
    Vinv = np.linalg.inv(V)
    Bt = Vinv @ (B_re.astype(np.float64) + 1j * B_im.astype(np.float64))  # [N, IN]
    Ct = (C_re.astype(np.float64) + 1j * C_im.astype(np.float64)) @ V     # [OUT, N]
    rho_all = np.abs(w)
    order = np.argsort(-rho_all)
    li, si = order[:NL], order[NL:]

    # long modes: balance ||bt_n|| = 1
    bn = np.linalg.norm(Bt[li], axis=1)
    Btl = Bt[li] / bn[:, None]
    Ctl = Ct[:, li] * bn[None, :]
    wl = w[li]
    rho = np.abs(wl)
    theta = np.angle(wl)

    # conv kernels for short modes (+ D at lag 0), lhsT layout [IN, OUT]
    ws = w[si]
    convs = []
    for k in range(KCONV):
        M = ((Ct[:, si] * (ws ** k)) @ Bt[si]).real
        if k == 0:
            M = M + D_w.astype(np.float64)
        convs.append(np.ascontiguousarray(M.T, dtype=np.float16))  # [IN, OUT]

    tg = np.arange(1, T + 1, dtype=np.float64)
    ang = np.outer(theta, tg)  # [NL, T]
    cost = np.cos(ang).astype(np.float16).reshape(LT, 128, T)
    sint = np.sin(ang).astype(np.float16).reshape(LT, 128, T)
    rho_b = np.broadcast_to(
        rho.astype(np.float32).reshape(LT, 128, 1), (LT, 128, BLK)
    ).transpose(1, 0, 2).reshape(128, LT * BLK).copy()

    def tbpiece(k):
        cs = cost[:, :, k * BLK:(k + 1) * BLK]  # [LT, 128, BLK]
        ss = sint[:, :, k * BLK:(k + 1) * BLK]
        return [np.ascontiguousarray(cs.transpose(1, 0, 2).reshape(128, LT * BLK)),
                np.ascontiguousarray(ss.transpose(1, 0, 2).reshape(128, LT * BLK))]

    ctrT = np.ascontiguousarray(Ctl.real.T, dtype=np.float16)   # [NL, OUT]
    ctiT = np.ascontiguousarray(-Ctl.imag.T, dtype=np.float16)  # [NL, OUT]

    parts = [np.concatenate(convs, axis=1)]  # [128, KCONV*OUT]
    parts += [
        np.ascontiguousarray(Btl.real.T, dtype=np.float16),  # [IN, NL]
        np.ascontiguousarray(Btl.imag.T, dtype=np.float16),
    ]
    parts += tbpiece(0)
    parts += [np.ascontiguousarray(ctrT.reshape(LT, 128, OUT).transpose(1, 0, 2)
                                   .reshape(128, LT * OUT))]
    parts += [np.ascontiguousarray(ctiT.reshape(LT, 128, OUT).transpose(1, 0, 2)
                                   .reshape(128, LT * OUT))]
    for k in range(1, TBLK):
        parts += tbpiece(k)
    shared16 = np.concatenate(parts, axis=1)  # [128, BLOBW - W_UT] fp16
    return shared16, rho_b


def _ensure_axon_hooks():
    import types
    try:
        from antenv import axon_hooks  # noqa: F401
        return
    except ImportError:
        pass
    try:
        import antenv
        mod = types.ModuleType("antenv.axon_hooks")
        _hook = [None]
        mod.set_axon_ntff_profile_hook = lambda h: _hook.__setitem__(0, h)
        mod.get_axon_ntff_profile_hook = lambda: _hook[0]
        sys.modules["antenv.axon_hooks"] = mod
        antenv.axon_hooks = mod
        if "/root/.axon_site" not in sys.path:
            sys.path.insert(0, "/root/.axon_site")
        from trn_agent_boot.trn_boot import _ntff_profile_via_ctypes
        h = _ntff_profile_via_ctypes("/opt/axon/libaxon_pjrt.so")
        if h is not None:
            mod.set_axon_ntff_profile_hook(h)
    except Exception:
        pass


def kernel(u, A_re, A_im, B_re, B_im, C_re, C_im, D_w, output_bias):
    global LAST_RESULT, _NC_CACHE
    from concourse import bass_utils

    _ensure_axon_hooks()

    u = np.asarray(u, dtype=np.float32)
    shared16, rho_b = _host_prep(
        np.asarray(A_re), np.asarray(A_im), np.asarray(B_re), np.asarray(B_im),
        np.asarray(C_re), np.asarray(C_im), np.asarray(D_w)
    )

    if _NC_CACHE is None:
        _NC_CACHE = _build_nc()
    nc = _NC_CACHE

    in_maps = []
    for k in range(NCORES):
        u_pair = u[BLOCAL * k:BLOCAL * (k + 1)]  # [2, T, IN]
        ut = np.ascontiguousarray(
            u_pair.transpose(2, 0, 1).reshape(128, COLS)
        ).astype(np.float16)
        in_maps.append({
            "blob": np.concatenate([ut, shared16], axis=1),
            "rho": rho_b,
        })

    res = bass_utils.run_bass_kernel_spmd(nc, in_maps, core_ids=list(range(NCORES)))
    LAST_RESULT = res

    y = np.empty((BATCH, T, OUT), dtype=np.float32)
    for k in range(NCORES):
        yd = res.results[k]["y"].astype(np.float32)  # [OUT, COLS]
        y[BLOCAL * k:BLOCAL * (k + 1)] = (
            yd.reshape(OUT, BLOCAL, T).transpose(1, 2, 0)
        )
    y += np.asarray(output_bias, dtype=np.float32)
    return y


# revision 4
# speedup vs baseline: 2.6428x; 1.0641x over previous
"""Trainium2 Bass kernel for nn_BaseLinearSSM.

y[b,t] = Re(C @ x_{t+1}) + D @ u[b,t] + bias,  x_{t+1} = A x_t + B u_t  (complex A,B,C)

Strategy (v2, hybrid fp16):
  Host (fp64): eigendecompose A = V diag(w) V^-1, fold V into B/C:
  Bt = V^-1 B, Ct = C V.  Sort modes by |w| descending and split:

  * LONG modes (top NL=256, two 128-partition tiles, per-mode balanced so
    ||bt_n|| = 1): modulated real scans exactly as v1 --
    f = Bt^T u -> g = e^{-i th t} f -> tensor_tensor_scan(rho) -> demod
    products -> y matmuls.  All elementwise tensors are fp16 so DVE runs
    in 2x mode; scans keep an fp32 rho and internal fp32 state.

  * SHORT modes (|w| < ~0.64): their memory is ~16 steps, so their whole
    contribution (plus the D term at lag 0) is a 16-lag convolution
    y += sum_k Re(Ct w^k Bt) u_{t-k}, computed entirely on the tensor
    engine with fp16 [128,128] kernels accumulating into the y PSUM bank.

  Numerics validated host-side: fp16 hybrid rel err ~3e-3 (gate 2e-2).

  Cores are fully independent (weights replicated); host shards u (2 of
  16 batch per core) and gathers y (fp16 out, converted + bias on host).
"""

import sys

import numpy as np

if "/opt/trn_rl_repo" not in sys.path:
    sys.path.insert(0, "/opt/trn_rl_repo")

BATCH, T, IN, OUT, N = 16, 2048, 128, 128, 512
NCORES = 8
BLOCAL = BATCH // NCORES  # 2
COLS = BLOCAL * T         # 4096 columns per core, col = b*T + t
BLK = 512                 # columns per pipeline block
NBLK = COLS // BLK        # 8 blocks
TBLK = T // BLK           # 4 t-blocks per batch element
NL = 256                  # long modes (scan path)
LT = NL // 128            # 2 long-mode partition tiles
KCONV = 16                # conv lags for short modes (incl. D at lag 0)

# fp16 blob layout (widths in fp16 elements), issue order = first-use order:
#   ut | convW | btr | bti | tb0(c|s) | ctrT | ctiT | tb1 | tb2 | tb3
W_UT = COLS
W_CONV = KCONV * OUT
W_B = LT * 128
TBW = 2 * LT * BLK        # one tb's cos+sin tables
W_CT = LT * OUT
RHOW = LT * BLK           # fp16 rho table width (scan multiplier)
BLOBW = W_UT + W_CONV + 2 * W_B + 3 * W_CT + RHOW + TBLK * TBW

# engine assignment for elementwise sites (tune): 'v' = vector, 'g' = gpsimd
MOD_ENGINES = ["v", "v", "v", "v", "v", "v"]   # t1,t2,gr,t3,t4,gi
SCAN_ENGINES = ["v", "v", "v", "v"]            # zr0, zi0, zr1, zi1
DEM_ENGINES = ["v", "v", "v", "v"]             # p,q,r,w2

LAST_RESULT = None
_NC_CACHE = None


def _build_nc():
    from concourse import bass, mybir
    from concourse import tile

    f16 = mybir.dt.float16
    f32 = mybir.dt.float32
    op = mybir.AluOpType

    nc = bass.Bass("TRN2", target_bir_lowering=False, debug=False)

    blob = nc.dram_tensor("blob", [128, BLOBW], f16, kind="ExternalInput")
    yout = nc.dram_tensor("y", [OUT, COLS], f16, kind="ExternalOutput")

    with tile.TileContext(nc) as tc:
        with (
            tc.tile_pool(name="const", bufs=1) as cpool,
            tc.tile_pool(name="f16", bufs=2) as fspool,
            tc.tile_pool(name="tmp", bufs=2) as tpool,
            tc.tile_pool(name="gp", bufs=2) as gpool,
            tc.tile_pool(name="zp", bufs=2) as zpool,
            tc.tile_pool(name="dm", bufs=2) as dpool,
            tc.tile_pool(name="ysb", bufs=2) as spool,
            tc.tile_pool(name="fps", bufs=1, space="PSUM") as fpool,
            tc.tile_pool(name="yps", bufs=2, space="PSUM") as ypool,
        ):
            blob_sb = cpool.tile([128, BLOBW], f16)
            # chunked DMA in first-use order
            bounds = [0, W_UT + W_CONV, W_UT + W_CONV + 2 * W_B]
            bounds.append(bounds[-1] + TBW + RHOW)
            bounds.append(bounds[-1] + 3 * W_CT)
            for _ in range(1, TBLK):
                bounds.append(bounds[-1] + TBW)
            for a, bnd in zip(bounds[:-1], bounds[1:]):
                nc.sync.dma_start(blob_sb[:, a:bnd], blob[:, a:bnd])

            o = [0]
            def take(w):
                s = blob_sb[:, o[0]:o[0] + w]
                o[0] += w
                return s
            ut_sb = take(W_UT)
            convw = take(W_CONV)
            btr_sb = take(W_B)
            bti_sb = take(W_B)
            ct_tb = [None] * TBLK
            st_tb = [None] * TBLK
            ct_tb[0] = take(LT * BLK)
            st_tb[0] = take(LT * BLK)
            rho_sb = take(RHOW)
            ctr_sb = take(W_CT)
            ctrn_sb = take(W_CT)
            cti_sb = take(W_CT)
            for k in range(1, TBLK):
                ct_tb[k] = take(LT * BLK)
                st_tb[k] = take(LT * BLK)
            assert o[0] == BLOBW

            eng = {"v": nc.vector, "g": nc.gpsimd}

            def emit_conv_f(j):
                """PE: conv matmuls + f matmuls for block j; returns psum tiles."""
                b, tb = divmod(j, TBLK)
                col0 = b * T + tb * BLK
                yps = ypool.tile([128, BLK], f32, tag="y")
                for k in range(KCONV):
                    wk = convw[:, k * OUT:(k + 1) * OUT]
                    if tb == 0 and k > 0:
                        nc.tensor.matmul(
                            yps[:, k:BLK], wk, ut_sb[:, col0:col0 + BLK - k],
                            start=False, stop=False,
                        )
                    else:
                        nc.tensor.matmul(
                            yps[:], wk, ut_sb[:, col0 - k:col0 + BLK - k],
                            start=(k == 0), stop=False,
                        )
                ucols = ut_sb[:, col0:col0 + BLK]
                fps = []
                for m in range(LT):
                    fre = fpool.tile([128, BLK], f32, tag=f"fr{m}")
                    fim = fpool.tile([128, BLK], f32, tag=f"fi{m}")
                    nc.tensor.matmul(fre[:], btr_sb[:, m * 128:(m + 1) * 128], ucols)
                    nc.tensor.matmul(fim[:], bti_sb[:, m * 128:(m + 1) * 128], ucols)
                    fps.append((fre, fim))
                return yps, ucols, fps

            def emit_evict_f(fps):
                """ACT: evict f PSUM -> one wide fp16 SBUF pair."""
                fr16 = fspool.tile([128, LT * BLK], f16, tag="fr16")
                fi16 = fspool.tile([128, LT * BLK], f16, tag="fi16")
                for m, (fre, fim) in enumerate(fps):
                    nc.scalar.copy(fr16[:, m * BLK:(m + 1) * BLK], fre[:])
                    nc.scalar.copy(fi16[:, m * BLK:(m + 1) * BLK], fim[:])
                return fr16, fi16

            zprev = [None, None]  # (zr, zi) wide tiles of previous block

            def emit_dve(j, fr16, fi16):
                b, tb = divmod(j, TBLK)
                ctt, stt = ct_tb[tb][:], st_tb[tb][:]
                W = LT * BLK
                # modulate (wide over both tiles)
                t1 = tpool.tile([128, W], f16, tag="t1")
                t2 = tpool.tile([128, W], f16, tag="t2")
                gr = gpool.tile([128, W], f16, tag="gr")
                eng[MOD_ENGINES[0]].tensor_tensor(t1[:], ctt, fr16[:], op=op.mult)
                eng[MOD_ENGINES[1]].tensor_tensor(t2[:], stt, fi16[:], op=op.mult)
                eng[MOD_ENGINES[2]].tensor_tensor(gr[:], t1[:], t2[:], op=op.add)
                t3 = tpool.tile([128, W], f16, tag="t3")
                t4 = tpool.tile([128, W], f16, tag="t4")
                gi = gpool.tile([128, W], f16, tag="gi")
                eng[MOD_ENGINES[3]].tensor_tensor(t3[:], ctt, fi16[:], op=op.mult)
                eng[MOD_ENGINES[4]].tensor_tensor(t4[:], stt, fr16[:], op=op.mult)
                eng[MOD_ENGINES[5]].tensor_tensor(gi[:], t3[:], t4[:], op=op.subtract)
                # scans (per tile, chained across tb within each batch elem)
                zr = zpool.tile([128, W], f16, tag="zr")
                zi = zpool.tile([128, W], f16, tag="zi")
                for m in range(LT):
                    sl = slice(m * BLK, (m + 1) * BLK)
                    init_r = 0.0 if tb == 0 else zprev[0][:, m * BLK + BLK - 1:m * BLK + BLK]
                    init_i = 0.0 if tb == 0 else zprev[1][:, m * BLK + BLK - 1:m * BLK + BLK]
                    eng[SCAN_ENGINES[2 * m]].tensor_tensor_scan(
                        zr[:, sl], rho_sb[:, sl], gr[:, sl], init_r,
                        op0=op.mult, op1=op.add,
                    )
                    eng[SCAN_ENGINES[2 * m + 1]].tensor_tensor_scan(
                        zi[:, sl], rho_sb[:, sl], gi[:, sl], init_i,
                        op0=op.mult, op1=op.add,
                    )
                zprev[0], zprev[1] = zr, zi
                # demod products
                p = dpool.tile([128, W], f16, tag="p")
                q = dpool.tile([128, W], f16, tag="q")
                r = dpool.tile([128, W], f16, tag="r")
                w2 = dpool.tile([128, W], f16, tag="w2")
                eng[DEM_ENGINES[0]].tensor_tensor(p[:], ctt, zr[:], op=op.mult)
                eng[DEM_ENGINES[1]].tensor_tensor(q[:], stt, zr[:], op=op.mult)
                eng[DEM_ENGINES[2]].tensor_tensor(r[:], stt, zi[:], op=op.mult)
                eng[DEM_ENGINES[3]].tensor_tensor(w2[:], ctt, zi[:], op=op.mult)
                return p, q, r, w2

            def emit_y(j, yps, p, q, r, w2):
                b, tb = divmod(j, TBLK)
                col0 = b * T + tb * BLK
                # y += Ctr.p + Cti.q + (-Ctr).r + Cti.w2
                for m in range(LT):
                    sl = slice(m * BLK, (m + 1) * BLK)
                    ctr = ctr_sb[:, m * OUT:(m + 1) * OUT]
                    ctrn = ctrn_sb[:, m * OUT:(m + 1) * OUT]
                    cti = cti_sb[:, m * OUT:(m + 1) * OUT]
                    nc.tensor.matmul(yps[:], ctr, p[:, sl], start=False, stop=False)
                    nc.tensor.matmul(yps[:], cti, q[:, sl], start=False, stop=False)
                    nc.tensor.matmul(yps[:], ctrn, r[:, sl], start=False, stop=False)
                    nc.tensor.matmul(
                        yps[:], cti, w2[:, sl], start=False, stop=(m == LT - 1)
                    )
                ysb = spool.tile([128, BLK], f16, tag="ysb")
                nc.scalar.copy(ysb[:], yps[:])
                nc.gpsimd.dma_start(yout[:, col0:col0 + BLK], ysb[:])

            # software-pipelined emission
            yps_c, uc_c, fps_c = emit_conv_f(0)
            ev_c = emit_evict_f(fps_c)
            for j in range(NBLK):
                if j + 1 < NBLK:
                    yps_n, uc_n, fps_n = emit_conv_f(j + 1)
                    ev_n = emit_evict_f(fps_n)
                prods = emit_dve(j, *ev_c)
                emit_y(j, yps_c, *prods)
                if j + 1 < NBLK:
                    yps_c, uc_c, fps_c = yps_n, uc_n, fps_n
                    ev_c = ev_n

    _legalize_multi_waits(nc)
    return nc


def _legalize_multi_waits(nc):
    """This walrus build accepts a single sync wait per instruction; split
    any multi-wait instruction into same-engine single-wait NoOps + the
    original carrying the last wait (program order chains them)."""
    import bass_rust
    from concourse import mybir

    uid = [0]
    for fn in nc.m.functions:
        for bb in fn.blocks:
            insts = bb.instructions
            new = []
            changed = False
            for inst in insts:
                si = inst.sync_info
                if si is not None and len(si.on_wait) > 1:
                    waits = list(si.on_wait)
                    for w in waits[:-1]:
                        uid[0] += 1
                        new.append(mybir.InstNoOp(
                            name=f"mwsplit-{uid[0]}",
                            engine=inst.engine,
                            ins=[], outs=[],
                            sync_info=bass_rust.SyncInfo(on_wait=[w], on_update=[]),
                        ))
                    inst.sync_info = bass_rust.SyncInfo(
                        on_wait=[waits[-1]], on_update=list(si.on_update)
                    )
                    changed = True
                new.append(inst)
            if changed:
                bb.instructions = new


def _host_prep(A_re, A_im, B_re, B_im, C_re, C_im, D_w):
    """fp64 eigendecomposition, mode sort/split, fp16 table/weight layouts."""
    A = A_re.astype(np.float64) + 1j * A_im.astype(np.float64)
    w, V = np.linalg.eig(A)
    Vinv = np.linalg.inv(V)
    Bt = Vinv @ (B_re.astype(np.float64) + 1j * B_im.astype(np.float64))  # [N, IN]
    Ct = (C_re.astype(np.float64) + 1j * C_im.astype(np.float64)) @ V     # [OUT, N]
    rho_all = np.abs(w)
    order = np.argsort(-rho_all)
    li, si = order[:NL], order[NL:]

    # long modes: balance ||bt_n|| = 1
    bn = np.linalg.norm(Bt[li], axis=1)
    Btl = Bt[li] / bn[:, None]
    Ctl = Ct[:, li] * bn[None, :]
    wl = w[li]
    rho = np.abs(wl)
    theta = np.angle(wl)

    # conv kernels for short modes (+ D at lag 0), lhsT layout [IN, OUT]
    ws = w[si]
    convs = []
    for k in range(KCONV):
        M = ((Ct[:, si] * (ws ** k)) @ Bt[si]).real
        if k == 0:
            M = M + D_w.astype(np.float64)
        convs.append(np.ascontiguousarray(M.T, dtype=np.float16))  # [IN, OUT]

    tg = np.arange(1, T + 1, dtype=np.float64)
    ang = np.outer(theta, tg)  # [NL, T]
    cost = np.cos(ang).astype(np.float16).reshape(LT, 128, T)
    sint = np.sin(ang).astype(np.float16).reshape(LT, 128, T)
    rho_b = np.broadcast_to(
        rho.astype(np.float16).reshape(LT, 128, 1), (LT, 128, BLK)
    ).transpose(1, 0, 2).reshape(128, LT * BLK).copy()

    def tbpiece(k):
        cs = cost[:, :, k * BLK:(k + 1) * BLK]  # [LT, 128, BLK]
        ss = sint[:, :, k * BLK:(k + 1) * BLK]
        return [np.ascontiguousarray(cs.transpose(1, 0, 2).reshape(128, LT * BLK)),
                np.ascontiguousarray(ss.transpose(1, 0, 2).reshape(128, LT * BLK))]

    ctrT = np.ascontiguousarray(Ctl.real.T, dtype=np.float16)   # [NL, OUT]
    ctiT = np.ascontiguousarray(-Ctl.imag.T, dtype=np.float16)  # [NL, OUT]

    parts = [np.concatenate(convs, axis=1)]  # [128, KCONV*OUT]
    parts += [
        np.ascontiguousarray(Btl.real.T, dtype=np.float16),  # [IN, NL]
        np.ascontiguousarray(Btl.imag.T, dtype=np.float16),
    ]
    parts += tbpiece(0)
    parts += [rho_b]
    for cc in (ctrT, -ctrT, ctiT):
        parts += [np.ascontiguousarray(
            np.asarray(cc, dtype=np.float16).reshape(LT, 128, OUT)
            .transpose(1, 0, 2).reshape(128, LT * OUT))]
    for k in range(1, TBLK):
        parts += tbpiece(k)
    shared16 = np.concatenate(parts, axis=1)  # [128, BLOBW - W_UT] fp16
    return shared16


def _ensure_axon_hooks():
    import types
    try:
        from antenv import axon_hooks  # noqa: F401
        return
    except ImportError:
        pass
    try:
        import antenv
        mod = types.ModuleType("antenv.axon_hooks")
        _hook = [None]
        mod.set_axon_ntff_profile_hook = lambda h: _hook.__setitem__(0, h)
        mod.get_axon_ntff_profile_hook = lambda: _hook[0]
        sys.modules["antenv.axon_hooks"] = mod
        antenv.axon_hooks = mod
        if "/root/.axon_site" not in sys.path:
            sys.path.insert(0, "/root/.axon_site")
        from trn_agent_boot.trn_boot import _ntff_profile_via_ctypes
        h = _ntff_profile_via_ctypes("/opt/axon/libaxon_pjrt.so")
        if h is not None:
            mod.set_axon_ntff_profile_hook(h)
    except Exception:
        pass


def kernel(u, A_re, A_im, B_re, B_im, C_re, C_im, D_w, output_bias):
    global LAST_RESULT, _NC_CACHE
    from concourse import bass_utils

    _ensure_axon_hooks()

    u = np.asarray(u, dtype=np.float32)
    shared16 = _host_prep(
        np.asarray(A_re), np.asarray(A_im), np.asarray(B_re), np.asarray(B_im),
        np.asarray(C_re), np.asarray(C_im), np.asarray(D_w)
    )

    if _NC_CACHE is None:
        _NC_CACHE = _build_nc()
    nc = _NC_CACHE

    in_maps = []
    for k in range(NCORES):
        u_pair = u[BLOCAL * k:BLOCAL * (k + 1)]  # [2, T, IN]
        ut = np.ascontiguousarray(
            u_pair.transpose(2, 0, 1).reshape(128, COLS)
        ).astype(np.float16)
        in_maps.append({"blob": np.concatenate([ut, shared16], axis=1)})

    res = bass_utils.run_bass_kernel_spmd(nc, in_maps, core_ids=list(range(NCORES)))
    LAST_RESULT = res

    y = np.empty((BATCH, T, OUT), dtype=np.float32)
    for k in range(NCORES):
        yd = res.results[k]["y"].astype(np.float32)  # [OUT, COLS]
        y[BLOCAL * k:BLOCAL * (k + 1)] = (
            yd.reshape(OUT, BLOCAL, T).transpose(1, 2, 0)
        )
    y += np.asarray(output_bias, dtype=np.float32)
    return y


# revision 8
# speedup vs baseline: 3.2678x; 1.2365x over previous
"""Trainium2 Bass kernel for nn_BaseLinearSSM.

y[b,t] = Re(C @ x_{t+1}) + D @ u[b,t] + bias,  x_{t+1} = A x_t + B u_t  (complex A,B,C)

Strategy (v2, hybrid fp16):
  Host (fp64): eigendecompose A = V diag(w) V^-1, fold V into B/C:
  Bt = V^-1 B, Ct = C V.  Sort modes by |w| descending and split:

  * LONG modes (top NL=256, two 128-partition tiles, per-mode balanced so
    ||bt_n|| = 1): modulated real scans exactly as v1 --
    f = Bt^T u -> g = e^{-i th t} f -> tensor_tensor_scan(rho) -> demod
    products -> y matmuls.  All elementwise tensors are fp16 so DVE runs
    in 2x mode; scans keep an fp32 rho and internal fp32 state.

  * SHORT modes (|w| < ~0.64): their memory is ~16 steps, so their whole
    contribution (plus the D term at lag 0) is a 16-lag convolution
    y += sum_k Re(Ct w^k Bt) u_{t-k}, computed entirely on the tensor
    engine with fp16 [128,128] kernels accumulating into the y PSUM bank.

  Numerics validated host-side: fp16 hybrid rel err ~3e-3 (gate 2e-2).

  Cores are fully independent (weights replicated); host shards u (2 of
  16 batch per core) and gathers y (fp16 out, converted + bias on host).
"""

import sys

import numpy as np

if "/opt/trn_rl_repo" not in sys.path:
    sys.path.insert(0, "/opt/trn_rl_repo")

BATCH, T, IN, OUT, N = 16, 2048, 128, 128, 512
NCORES = 8
BLOCAL = BATCH // NCORES  # 2
COLS = BLOCAL * T         # 4096 columns per core, col = b*T + t
BLK = 512                 # columns per pipeline block
NBLK = COLS // BLK        # 8 blocks
TBLK = T // BLK           # 4 t-blocks per batch element
NL = 256                  # long modes (scan path)
LT = NL // 128            # 2 long-mode partition tiles
KCONV = 16                # conv lags for short modes (incl. D at lag 0)

# fp16 blob layout (widths in fp16 elements), issue order = first-use order:
#   ut | convW | btr | bti | tb0(c|s) | ctrT | ctiT | tb1 | tb2 | tb3
W_UT = COLS
W_CONV = KCONV * OUT
W_B = LT * 128
TBW = 2 * LT * BLK        # one tb's cos+sin tables
W_CT = LT * OUT
RHOW = LT * BLK           # fp16 rho table width (scan multiplier)
BLOBW = W_UT + W_CONV + 2 * W_B + 3 * W_CT + RHOW + TBLK * TBW

# engine assignment for elementwise sites (tune): 'v' = vector, 'g' = gpsimd
MOD_ENGINES = ["v", "v", "v", "v", "v", "v"]   # t1,t2,gr,t3,t4,gi
SCAN_ENGINES = ["v", "v", "v", "v"]            # zr0, zi0, zr1, zi1
DEM_ENGINES = ["v", "v", "v", "v"]             # p,q,r,w2

LAST_RESULT = None
_NC_CACHE = None


def _build_nc():
    from concourse import bass, mybir
    from concourse import tile

    f16 = mybir.dt.float16
    f32 = mybir.dt.float32
    op = mybir.AluOpType

    nc = bass.Bass("TRN2", target_bir_lowering=False, debug=False)

    blob = nc.dram_tensor("blob", [128, BLOBW], f16, kind="ExternalInput")
    yout = nc.dram_tensor("y", [OUT, COLS], f16, kind="ExternalOutput")

    with tile.TileContext(nc) as tc:
        with (
            tc.tile_pool(name="const", bufs=1) as cpool,
            tc.tile_pool(name="f16", bufs=3) as fspool,
            tc.tile_pool(name="tmp", bufs=2) as tpool,
            tc.tile_pool(name="gp", bufs=2) as gpool,
            tc.tile_pool(name="zp", bufs=2) as zpool,
            tc.tile_pool(name="dm", bufs=2) as dpool,
            tc.tile_pool(name="ysb", bufs=2) as spool,
            tc.tile_pool(name="fps", bufs=1, space="PSUM") as fpool,
            tc.tile_pool(name="yps", bufs=3, space="PSUM") as ypool,
        ):
            blob_sb = cpool.tile([128, BLOBW], f16)
            # chunked DMA in first-use order
            bounds = [0, W_UT + W_CONV, W_UT + W_CONV + 2 * W_B]
            bounds.append(bounds[-1] + TBW + RHOW)
            bounds.append(bounds[-1] + 3 * W_CT)
            for _ in range(1, TBLK):
                bounds.append(bounds[-1] + TBW)
            for a, bnd in zip(bounds[:-1], bounds[1:]):
                nc.sync.dma_start(blob_sb[:, a:bnd], blob[:, a:bnd])

            o = [0]
            def take(w):
                s = blob_sb[:, o[0]:o[0] + w]
                o[0] += w
                return s
            ut_sb = take(W_UT)
            convw = take(W_CONV)
            btr_sb = take(W_B)
            bti_sb = take(W_B)
            ct_tb = [None] * TBLK
            st_tb = [None] * TBLK
            ct_tb[0] = take(LT * BLK)
            st_tb[0] = take(LT * BLK)
            rho_sb = take(RHOW)
            ctr_sb = take(W_CT)
            ctrn_sb = take(W_CT)
            cti_sb = take(W_CT)
            for k in range(1, TBLK):
                ct_tb[k] = take(LT * BLK)
                st_tb[k] = take(LT * BLK)
            assert o[0] == BLOBW

            eng = {"v": nc.vector, "g": nc.gpsimd}

            def emit_conv_f(j):
                """PE: conv matmuls + f matmuls for block j; returns psum tiles."""
                b, tb = divmod(j, TBLK)
                col0 = b * T + tb * BLK
                # f matmuls first: they gate the ACT evictions -> DVE chain
                ucols = ut_sb[:, col0:col0 + BLK]
                fps = []
                for m in range(LT):
                    fre = fpool.tile([128, BLK], f32, tag=f"fr{m}")
                    fim = fpool.tile([128, BLK], f32, tag=f"fi{m}")
                    nc.tensor.matmul(fre[:], btr_sb[:, m * 128:(m + 1) * 128], ucols)
                    nc.tensor.matmul(fim[:], bti_sb[:, m * 128:(m + 1) * 128], ucols)
                    fps.append((fre, fim))
                yps = ypool.tile([128, BLK], f32, tag="y")
                for k in range(KCONV):
                    wk = convw[:, k * OUT:(k + 1) * OUT]
                    if tb == 0 and k > 0:
                        nc.tensor.matmul(
                            yps[:, k:BLK], wk, ut_sb[:, col0:col0 + BLK - k],
                            start=False, stop=False,
                        )
                    else:
                        nc.tensor.matmul(
                            yps[:], wk, ut_sb[:, col0 - k:col0 + BLK - k],
                            start=(k == 0), stop=False,
                        )
                return yps, ucols, fps

            def emit_evict_f(fps):
                """ACT: evict f PSUM -> one wide fp16 SBUF pair."""
                fr16 = fspool.tile([128, LT * BLK], f16, tag="fr16")
                fi16 = fspool.tile([128, LT * BLK], f16, tag="fi16")
                for m, (fre, fim) in enumerate(fps):
                    nc.scalar.copy(fr16[:, m * BLK:(m + 1) * BLK], fre[:])
                    nc.scalar.copy(fi16[:, m * BLK:(m + 1) * BLK], fim[:])
                return fr16, fi16

            zprev = [None, None]  # (zr, zi) wide tiles of previous block

            def emit_dve(j, fr16, fi16):
                b, tb = divmod(j, TBLK)
                ctt, stt = ct_tb[tb][:], st_tb[tb][:]
                W = LT * BLK
                # modulate (wide over both tiles)
                t1 = tpool.tile([128, W], f16, tag="t1")
                t2 = tpool.tile([128, W], f16, tag="t2")
                gr = gpool.tile([128, W], f16, tag="gr")
                eng[MOD_ENGINES[0]].tensor_tensor(t1[:], ctt, fr16[:], op=op.mult)
                eng[MOD_ENGINES[1]].tensor_tensor(t2[:], stt, fi16[:], op=op.mult)
                eng[MOD_ENGINES[2]].tensor_tensor(gr[:], t1[:], t2[:], op=op.add)
                t3 = tpool.tile([128, W], f16, tag="t3")
                t4 = tpool.tile([128, W], f16, tag="t4")
                gi = gpool.tile([128, W], f16, tag="gi")
                eng[MOD_ENGINES[3]].tensor_tensor(t3[:], ctt, fi16[:], op=op.mult)
                eng[MOD_ENGINES[4]].tensor_tensor(t4[:], stt, fr16[:], op=op.mult)
                eng[MOD_ENGINES[5]].tensor_tensor(gi[:], t3[:], t4[:], op=op.subtract)
                # scans (per tile, chained across tb within each batch elem)
                zr = zpool.tile([128, W], f16, tag="zr")
                zi = zpool.tile([128, W], f16, tag="zi")
                for m in range(LT):
                    sl = slice(m * BLK, (m + 1) * BLK)
                    init_r = 0.0 if tb == 0 else zprev[0][:, m * BLK + BLK - 1:m * BLK + BLK]
                    init_i = 0.0 if tb == 0 else zprev[1][:, m * BLK + BLK - 1:m * BLK + BLK]
                    eng[SCAN_ENGINES[2 * m]].tensor_tensor_scan(
                        zr[:, sl], rho_sb[:, sl], gr[:, sl], init_r,
                        op0=op.mult, op1=op.add,
                    )
                    eng[SCAN_ENGINES[2 * m + 1]].tensor_tensor_scan(
                        zi[:, sl], rho_sb[:, sl], gi[:, sl], init_i,
                        op0=op.mult, op1=op.add,
                    )
                zprev[0], zprev[1] = zr, zi
                # demod products
                p = dpool.tile([128, W], f16, tag="p")
                q = dpool.tile([128, W], f16, tag="q")
                r = dpool.tile([128, W], f16, tag="r")
                w2 = dpool.tile([128, W], f16, tag="w2")
                eng[DEM_ENGINES[0]].tensor_tensor(p[:], ctt, zr[:], op=op.mult)
                eng[DEM_ENGINES[1]].tensor_tensor(q[:], stt, zr[:], op=op.mult)
                eng[DEM_ENGINES[2]].tensor_tensor(r[:], stt, zi[:], op=op.mult)
                eng[DEM_ENGINES[3]].tensor_tensor(w2[:], ctt, zi[:], op=op.mult)
                return p, q, r, w2

            def emit_y(j, yps, p, q, r, w2):
                b, tb = divmod(j, TBLK)
                col0 = b * T + tb * BLK
                # y += Ctr.p + Cti.q + (-Ctr).r + Cti.w2
                for m in range(LT):
                    sl = slice(m * BLK, (m + 1) * BLK)
                    ctr = ctr_sb[:, m * OUT:(m + 1) * OUT]
                    ctrn = ctrn_sb[:, m * OUT:(m + 1) * OUT]
                    cti = cti_sb[:, m * OUT:(m + 1) * OUT]
                    nc.tensor.matmul(yps[:], ctr, p[:, sl], start=False, stop=False)
                    nc.tensor.matmul(yps[:], cti, q[:, sl], start=False, stop=False)
                    nc.tensor.matmul(yps[:], ctrn, r[:, sl], start=False, stop=False)
                    nc.tensor.matmul(
                        yps[:], cti, w2[:, sl], start=False, stop=(m == LT - 1)
                    )
                ysb = spool.tile([128, BLK], f16, tag="ysb")
                nc.scalar.copy(ysb[:], yps[:])
                nc.gpsimd.dma_start(yout[:, col0:col0 + BLK], ysb[:])

            # software-pipelined emission, lookahead 2 on the PE/ACT streams
            stage = []
            for j in range(min(2, NBLK)):
                yps_j, _, fps_j = emit_conv_f(j)
                stage.append((yps_j, emit_evict_f(fps_j)))
            for j in range(NBLK):
                prods = emit_dve(j, *stage[j][1])
                if j + 2 < NBLK:
                    yps_n, _, fps_n = emit_conv_f(j + 2)
                    stage.append((yps_n, emit_evict_f(fps_n)))
                emit_y(j, stage[j][0], *prods)

    _legalize_multi_waits(nc)
    return nc


def _legalize_multi_waits(nc):
    """This walrus build accepts a single sync wait per instruction; split
    any multi-wait instruction into same-engine single-wait NoOps + the
    original carrying the last wait (program order chains them)."""
    import bass_rust
    from concourse import mybir

    uid = [0]
    for fn in nc.m.functions:
        for bb in fn.blocks:
            insts = bb.instructions
            new = []
            changed = False
            for inst in insts:
                si = inst.sync_info
                if si is not None and len(si.on_wait) > 1:
                    waits = list(si.on_wait)
                    for w in waits[:-1]:
                        uid[0] += 1
                        new.append(mybir.InstNoOp(
                            name=f"mwsplit-{uid[0]}",
                            engine=inst.engine,
                            ins=[], outs=[],
                            sync_info=bass_rust.SyncInfo(on_wait=[w], on_update=[]),
                        ))
                    inst.sync_info = bass_rust.SyncInfo(
                        on_wait=[waits[-1]], on_update=list(si.on_update)
                    )
                    changed = True
                new.append(inst)
            if changed:
                bb.instructions = new


def _host_prep(A_re, A_im, B_re, B_im, C_re, C_im, D_w):
    """fp64 eigendecomposition, mode sort/split, fp16 table/weight layouts."""
    A = A_re.astype(np.float64) + 1j * A_im.astype(np.float64)
    w, V = np.linalg.eig(A)
    Vinv = np.linalg.inv(V)
    Bt = Vinv @ (B_re.astype(np.float64) + 1j * B_im.astype(np.float64))  # [N, IN]
    Ct = (C_re.astype(np.float64) + 1j * C_im.astype(np.float64)) @ V     # [OUT, N]
    rho_all = np.abs(w)
    order = np.argsort(-rho_all)
    li, si = order[:NL], order[NL:]

    # long modes: balance ||bt_n|| = 1
    bn = np.linalg.norm(Bt[li], axis=1)
    Btl = Bt[li] / bn[:, None]
    Ctl = Ct[:, li] * bn[None, :]
    wl = w[li]
    rho = np.abs(wl)
    theta = np.angle(wl)

    # conv kernels for short modes (+ D at lag 0), lhsT layout [IN, OUT]
    ws = w[si]
    convs = []
    for k in range(KCONV):
        M = ((Ct[:, si] * (ws ** k)) @ Bt[si]).real
        if k == 0:
            M = M + D_w.astype(np.float64)
        convs.append(np.ascontiguousarray(M.T, dtype=np.float16))  # [IN, OUT]

    tg = np.arange(1, T + 1, dtype=np.float64)
    ang = np.outer(theta, tg)  # [NL, T]
    cost = np.cos(ang).astype(np.float16).reshape(LT, 128, T)
    sint = np.sin(ang).astype(np.float16).reshape(LT, 128, T)
    rho_b = np.broadcast_to(
        rho.astype(np.float16).reshape(LT, 128, 1), (LT, 128, BLK)
    ).transpose(1, 0, 2).reshape(128, LT * BLK).copy()

    def tbpiece(k):
        cs = cost[:, :, k * BLK:(k + 1) * BLK]  # [LT, 128, BLK]
        ss = sint[:, :, k * BLK:(k + 1) * BLK]
        return [np.ascontiguousarray(cs.transpose(1, 0, 2).reshape(128, LT * BLK)),
                np.ascontiguousarray(ss.transpose(1, 0, 2).reshape(128, LT * BLK))]

    ctrT = np.ascontiguousarray(Ctl.real.T, dtype=np.float16)   # [NL, OUT]
    ctiT = np.ascontiguousarray(-Ctl.imag.T, dtype=np.float16)  # [NL, OUT]

    parts = [np.concatenate(convs, axis=1)]  # [128, KCONV*OUT]
    parts += [
        np.ascontiguousarray(Btl.real.T, dtype=np.float16),  # [IN, NL]
        np.ascontiguousarray(Btl.imag.T, dtype=np.float16),
    ]
    parts += tbpiece(0)
    parts += [rho_b]
    for cc in (ctrT, -ctrT, ctiT):
        parts += [np.ascontiguousarray(
            np.asarray(cc, dtype=np.float16).reshape(LT, 128, OUT)
            .transpose(1, 0, 2).reshape(128, LT * OUT))]
    for k in range(1, TBLK):
        parts += tbpiece(k)
    shared16 = np.concatenate(parts, axis=1)  # [128, BLOBW - W_UT] fp16
    return shared16


def _ensure_axon_hooks():
    import types
    try:
        from antenv import axon_hooks  # noqa: F401
        return
    except ImportError:
        pass
    try:
        import antenv
        mod = types.ModuleType("antenv.axon_hooks")
        _hook = [None]
        mod.set_axon_ntff_profile_hook = lambda h: _hook.__setitem__(0, h)
        mod.get_axon_ntff_profile_hook = lambda: _hook[0]
        sys.modules["antenv.axon_hooks"] = mod
        antenv.axon_hooks = mod
        if "/root/.axon_site" not in sys.path:
            sys.path.insert(0, "/root/.axon_site")
        from trn_agent_boot.trn_boot import _ntff_profile_via_ctypes
        h = _ntff_profile_via_ctypes("/opt/axon/libaxon_pjrt.so")
        if h is not None:
            mod.set_axon_ntff_profile_hook(h)
    except Exception:
        pass


def kernel(u, A_re, A_im, B_re, B_im, C_re, C_im, D_w, output_bias):
    global LAST_RESULT, _NC_CACHE
    from concourse import bass_utils

    _ensure_axon_hooks()

    u = np.asarray(u, dtype=np.float32)
    shared16 = _host_prep(
        np.asarray(A_re), np.asarray(A_im), np.asarray(B_re), np.asarray(B_im),
        np.asarray(C_re), np.asarray(C_im), np.asarray(D_w)
    )

    if _NC_CACHE is None:
        _NC_CACHE = _build_nc()
    nc = _NC_CACHE

    in_maps = []
    for k in range(NCORES):
        u_pair = u[BLOCAL * k:BLOCAL * (k + 1)]  # [2, T, IN]
        ut = np.ascontiguousarray(
            u_pair.transpose(2, 0, 1).reshape(128, COLS)
        ).astype(np.float16)
        in_maps.append({"blob": np.concatenate([ut, shared16], axis=1)})

    res = bass_utils.run_bass_kernel_spmd(nc, in_maps, core_ids=list(range(NCORES)))
    LAST_RESULT = res

    y = np.empty((BATCH, T, OUT), dtype=np.float32)
    for k in range(NCORES):
        yd = res.results[k]["y"].astype(np.float32)  # [OUT, COLS]
        y[BLOCAL * k:BLOCAL * (k + 1)] = (
            yd.reshape(OUT, BLOCAL, T).transpose(1, 2, 0)
        )
    y += np.asarray(output_bias, dtype=np.float32)
    return y
